# revision 8
# baseline (speedup 1.0000x reference)
"""BiLSTM-CRF Trainium2 kernel (8 NeuronCores).

Decomposition:
  K1 (cores 0-3 = forward LSTM, cores 4-7 = backward LSTM; each core owns a
      batch quarter of 16 examples): embedding gather -> selu input layer ->
      xp = x @ W_ih.T + b (spilled to DRAM, bf16) -> 512-step LSTM scan
      (weight-stationary orientation: gates come out transposed so h never
      needs an on-chip transpose). Backward direction runs as a forward scan
      on host-time-reversed inputs; padded steps are handled by masking the
      embeddings (zero state is a fixed point of the LSTM when xp == 0).
  K2 (8 cores, 8 examples each): logits = selu(o @ W_out.T + b_out) (masked),
      CRF log-partition via scaled forward+backward half-chains (interleaved
      to hide serial latency, periodic renorm), emission score via one-hot.
  Host: input slicing/swizzles/casts, h_b time-flip + regroup between K1 and
      K2, transition score (pure input gather), final loglik assembly.
"""

import json
import math

import numpy as np
import ml_dtypes

import concourse.bass as bass
import concourse.tile as tile
from concourse import mybir
from concourse import bass_utils
from concourse.masks import make_identity

F32 = mybir.dt.float32
BF16 = mybir.dt.bfloat16
I32 = mybir.dt.int32
AF = mybir.ActivationFunctionType
ALU = mybir.AluOpType

NF, B, T = 2, 64, 512
V, WD, H = 50000, 256, 512
NLAB = 48
L = NLAB + 2
START, STOP = L - 2, L - 1
NCORES = 8
BQ = B // 4            # examples per K1 core (batch quarter)
BE = B // 8            # examples per K2 core
NTOK = BQ * T          # tokens per K1 core (8192)
NCH = NTOK // 128      # 128-token chunks (64)
NSB = 4                # superblocks in phase 1
SB_TOK = NTOK // NSB   # tokens per superblock (2048)
SELU_A = 1.6732632423543772
SELU_S = 1.0507009873554805
SELU_SA = SELU_S * SELU_A
CLOG = 4.5             # CRF scaling constant (divide Ehat by e^CLOG per step)
S_HALF = T // 2

bf16 = lambda a: np.ascontiguousarray(a).astype(ml_dtypes.bfloat16)
f32 = lambda a: np.ascontiguousarray(a, dtype=np.float32)


# ---------------------------------------------------------------------------
# walrus on this image accepts at most ONE sync-wait per instruction; split
# extra waits onto NoOps inserted before the instruction (same engine).
# ---------------------------------------------------------------------------
def _split_multiwait(m: dict) -> int:
    ctr = 0
    for f in m.get("functions", []):
        for bb in f.get("blocks", []):
            new = []
            for inst in bb.get("instructions", []):
                si = inst.get("sync_info")
                waits = (si or {}).get("on_wait") or []
                if len(waits) > 1:
                    for w in waits[:-1]:
                        ctr += 1
                        new.append({
                            "debug": inst.get("debug", 0),
                            "engine": inst["engine"],
                            "ins": [], "outs": [],
                            "name": f"SWW-{ctr}",
                            "opcode": "NoOp",
                            "sync_info": {"on_update": [], "on_wait": [w]},
                        })
                    si["on_wait"] = [waits[-1]]
                new.append(inst)
            bb["instructions"] = new
    return ctr


class PatchedBass(bass.Bass):
    def to_json_bytes(self) -> bytes:
        raw = super().to_json_bytes()
        m = json.loads(raw)
        if _split_multiwait(m):
            raw = json.dumps(m).encode()
        return raw


# ---------------------------------------------------------------------------
# K1: embeddings -> input layer -> xp -> LSTM scan (one direction per core)
# ---------------------------------------------------------------------------
def build_k1():
    nc = PatchedBass(trn_type="TRN2")
    tok0 = nc.dram_tensor("tok0", [128, NCH], I32, kind="ExternalInput")
    tok1 = nc.dram_tensor("tok1", [128, NCH], I32, kind="ExternalInput")
    maskflat = nc.dram_tensor("maskflat", [NTOK], F32, kind="ExternalInput")
    emb0 = nc.dram_tensor("emb0", [V, WD], F32, kind="ExternalInput")
    emb1 = nc.dram_tensor("emb1", [V, WD], F32, kind="ExternalInput")
    WinT = nc.dram_tensor("WinT", [2 * WD, H], BF16, kind="ExternalInput")
    WihT = nc.dram_tensor("WihT", [H, 4 * H], BF16, kind="ExternalInput")
    WhhT = nc.dram_tensor("WhhT", [H, 4 * H], BF16, kind="ExternalInput")
    binT = nc.dram_tensor("binT", [128, 4], F32, kind="ExternalInput")
    bihT = nc.dram_tensor("bihT", [128, 16], F32, kind="ExternalInput")
    hsT = nc.dram_tensor("hsT", [4, 128, T, BQ], BF16, kind="ExternalOutput")

    with tile.TileContext(nc) as tc:
        with tc.tile_pool(name="weights", bufs=1) as wpool, \
             tc.tile_pool(name="dram", bufs=1, space="DRAM") as dpool:
            ident = wpool.tile([128, 128], F32)
            make_identity(nc, ident[:])
            win_sb = wpool.tile([128, 4, H], BF16)
            nc.sync.dma_start(win_sb[:], WinT[:].rearrange("(k p) m -> p k m", p=128))
            wih_sb = wpool.tile([128, 4, 4 * H], BF16)
            nc.sync.dma_start(wih_sb[:], WihT[:].rearrange("(k p) m -> p k m", p=128))
            whh_sb = wpool.tile([128, 4, 4 * H], BF16)
            nc.sync.dma_start(whh_sb[:], WhhT[:].rearrange("(k p) m -> p k m", p=128))
            bin_sb = wpool.tile([128, 4], F32)
            nc.sync.dma_start(bin_sb[:], binT[:])
            bih_sb = wpool.tile([128, 16], F32)
            nc.sync.dma_start(bih_sb[:], bihT[:])
            tok0_sb = wpool.tile([128, NCH], I32)
            nc.sync.dma_start(tok0_sb[:], tok0[:])
            tok1_sb = wpool.tile([128, NCH], I32)
            nc.sync.dma_start(tok1_sb[:], tok1[:])

            xpT_d = dpool.tile([16, 128, T, BQ], BF16)

            # ---------------- phase 1 ----------------
            with tc.tile_pool(name="gather", bufs=4) as gapool, \
                 tc.tile_pool(name="xemb", bufs=1) as xepool, \
                 tc.tile_pool(name="x1", bufs=1) as x1pool, \
                 tc.tile_pool(name="mch", bufs=3) as mpool, \
                 tc.tile_pool(name="tpsum", bufs=4, space="PSUM") as tppool, \
                 tc.tile_pool(name="p1psum", bufs=2, space="PSUM") as p1pool, \
                 tc.tile_pool(name="sl", bufs=3) as slpool, \
                 tc.tile_pool(name="xpst", bufs=3) as xppool:
                for sb in range(NSB):
                    base = sb * SB_TOK
                    nch = SB_TOK // 128  # 16 chunks per superblock
                    xe = xepool.tile([128, 4, SB_TOK], BF16)
                    for j in range(nch):
                        jg = sb * nch + j
                        g0 = gapool.tile([128, WD], F32, tag="g0")
                        g1 = gapool.tile([128, WD], F32, tag="g1")
                        nc.gpsimd.indirect_dma_start(
                            out=g0[:], out_offset=None, in_=emb0[:],
                            in_offset=bass.IndirectOffsetOnAxis(
                                ap=tok0_sb[:, jg:jg + 1], axis=0))
                        nc.gpsimd.indirect_dma_start(
                            out=g1[:], out_offset=None, in_=emb1[:],
                            in_offset=bass.IndirectOffsetOnAxis(
                                ap=tok1_sb[:, jg:jg + 1], axis=0))
                        mch = mpool.tile([128, 128], F32)
                        msrc = maskflat[:]
                        mask_bcast = bass.AP(
                            tensor=msrc.tensor, offset=base + j * 128,
                            ap=[[0, 128], [1, 128]])
                        nc.sync.dma_start(mch[:], mask_bcast)
                        for half in range(2):
                            for tb, gt in ((0, g0), (1, g1)):
                                tp = tppool.tile([128, 128], F32)
                                nc.tensor.transpose(
                                    tp[:], gt[:, half * 128:(half + 1) * 128],
                                    ident[:])
                                kk = 2 * tb + half
                                nc.vector.scalar_tensor_tensor(
                                    out=xe[:, kk, j * 128:(j + 1) * 128],
                                    in0=tp[:], scalar=1.0, in1=mch[:],
                                    op0=ALU.mult, op1=ALU.mult)
                    x1 = x1pool.tile([128, 4, SB_TOK], BF16)
                    for m in range(4):
                        for tsb in range(SB_TOK // 512):
                            ts_ = slice(tsb * 512, (tsb + 1) * 512)
                            ps = p1pool.tile([128, 512], F32, tag="ps1")
                            for k in range(4):
                                nc.tensor.matmul(
                                    ps[:], win_sb[:, k, m * 128:(m + 1) * 128],
                                    xe[:, k, ts_], start=(k == 0), stop=(k == 3))
                            bcol = bin_sb[:, m:m + 1]
                            tmin = slpool.tile([128, 512], F32, tag="tmin")
                            nc.vector.tensor_scalar(
                                out=tmin[:], in0=ps[:], scalar1=bcol, op0=ALU.add,
                                scalar2=0.0, op1=ALU.min)
                            pp = slpool.tile([128, 512], F32, tag="pp")
                            nc.vector.scalar_tensor_tensor(
                                out=pp[:], in0=ps[:], scalar=bcol, op0=ALU.add,
                                in1=tmin[:], op1=ALU.subtract)
                            ee = slpool.tile([128, 512], F32, tag="ee")
                            nc.scalar.activation(ee[:], tmin[:], AF.Exp)
                            nc.vector.tensor_scalar(
                                out=ee[:], in0=ee[:], scalar1=1.0, op0=ALU.subtract,
                                scalar2=SELU_SA, op1=ALU.mult)
                            nc.vector.scalar_tensor_tensor(
                                out=x1[:, m, ts_], in0=pp[:], scalar=SELU_S,
                                op0=ALU.mult, in1=ee[:], op1=ALU.add)
                    for m in range(16):
                        for tsb in range(SB_TOK // 512):
                            ts_ = slice(tsb * 512, (tsb + 1) * 512)
                            ps2 = p1pool.tile([128, 512], F32, tag="ps2")
                            for k in range(4):
                                nc.tensor.matmul(
                                    ps2[:], wih_sb[:, k, m * 128:(m + 1) * 128],
                                    x1[:, k, ts_], start=(k == 0), stop=(k == 3))
                            stage = xppool.tile([128, 512], BF16)
                            nc.scalar.activation(
                                stage[:], ps2[:], AF.Identity,
                                bias=bih_sb[:, m:m + 1])
                            t0 = (base + tsb * 512) // BQ
                            nc.sync.dma_start(
                                out=xpT_d[m, :, t0:t0 + 512 // BQ, :],
                                in_=stage[:].rearrange("p (t e) -> p t e", e=BQ))

            # ---------------- phase 2: LSTM scan ----------------
            with tc.tile_pool(name="pf", bufs=2) as pfpool, \
                 tc.tile_pool(name="gpsum", bufs=2, space="PSUM") as gppool, \
                 tc.tile_pool(name="gsb", bufs=2) as gsbpool, \
                 tc.tile_pool(name="acts", bufs=2) as actpool, \
                 tc.tile_pool(name="hc", bufs=2) as hcpool, \
                 tc.tile_pool(name="hst", bufs=2) as hstpool:
                h_prev = hcpool.tile([128, 4, BQ], BF16, tag="h")
                c_prev = hcpool.tile([128, 4, BQ], F32, tag="c")
                nc.vector.memset(h_prev[:], 0.0)
                nc.vector.memset(c_prev[:], 0.0)
                pf = None
                hstage = None
                PF_STEPS = 16
                for t in range(T):
                    if t % PF_STEPS == 0:
                        pf = pfpool.tile([128, 16, PF_STEPS, BQ], BF16)
                        nc.sync.dma_start(
                            pf[:], xpT_d[:, :, t:t + PF_STEPS, :]
                            .rearrange("m p t e -> p m t e"))
                    if t % 32 == 0:
                        hstage = hstpool.tile([128, 4, 32, BQ], BF16)
                    ps = gppool.tile([128, 16, BQ], F32)
                    for m in range(16):
                        for k in range(4):
                            nc.tensor.matmul(
                                ps[:, m, :], whh_sb[:, k, m * 128:(m + 1) * 128],
                                h_prev[:, k, :], start=(k == 0), stop=(k == 3))
                    g = gsbpool.tile([128, 16, BQ], F32)
                    nc.vector.tensor_add(
                        out=g[:, 0:12, :], in0=ps[:, 0:12, :],
                        in1=pf[:, 0:12, t % PF_STEPS, :])
                    nc.vector.tensor_add(
                        out=g[:, 12:16, :], in0=ps[:, 12:16, :],
                        in1=pf[:, 12:16, t % PF_STEPS, :])
                    sif = actpool.tile([128, 8, BQ], F32, tag="sif")
                    nc.scalar.activation(sif[:], g[:, 0:8, :], AF.Sigmoid)
                    tg = actpool.tile([128, 4, BQ], F32, tag="tg")
                    nc.scalar.activation(tg[:], g[:, 8:12, :], AF.Tanh)
                    so = actpool.tile([128, 4, BQ], F32, tag="so")
                    nc.scalar.activation(so[:], g[:, 12:16, :], AF.Sigmoid)
                    t1 = actpool.tile([128, 4, BQ], F32, tag="t1")
                    nc.vector.tensor_mul(t1[:], sif[:, 4:8, :], c_prev[:])
                    t2 = actpool.tile([128, 4, BQ], F32, tag="t2")
                    nc.vector.tensor_mul(t2[:], sif[:, 0:4, :], tg[:])
                    c_new = hcpool.tile([128, 4, BQ], F32, tag="c")
                    nc.vector.tensor_add(c_new[:], t1[:], t2[:])
                    th = actpool.tile([128, 4, BQ], F32, tag="th")
                    nc.scalar.activation(th[:], c_new[:], AF.Tanh)
                    h_new = hcpool.tile([128, 4, BQ], BF16, tag="h")
                    nc.vector.tensor_mul(h_new[:], so[:], th[:])
                    nc.gpsimd.tensor_copy(hstage[:, :, t % 32, :], h_new[:])
                    if t % 32 == 31:
                        nc.sync.dma_start(
                            out=hsT[:, :, t - 31:t + 1, :]
                            .rearrange("k p t e -> p k t e"),
                            in_=hstage[:])
                    h_prev, c_prev = h_new, c_new
    return nc


# ---------------------------------------------------------------------------
# K2: output layer + CRF + emission score (8 examples per core)
# ---------------------------------------------------------------------------
def build_k2():
    nc = PatchedBass(trn_type="TRN2")
    NT2 = T * BE  # tokens per core (4096)
    oT = nc.dram_tensor("oT", [8, 128, T, BE], BF16, kind="ExternalInput")
    WoutT = nc.dram_tensor("WoutT", [2 * H, L], BF16, kind="ExternalInput")
    boutT = nc.dram_tensor("boutT", [L, 1], F32, kind="ExternalInput")
    EhatTf = nc.dram_tensor("EhatTf", [L, L], F32, kind="ExternalInput")
    EhatB = nc.dram_tensor("EhatB", [L, L], F32, kind="ExternalInput")
    maskF = nc.dram_tensor("maskF", [S_HALF, BE], I32, kind="ExternalInput")
    maskB = nc.dram_tensor("maskB", [S_HALF, BE], I32, kind="ExternalInput")
    masktok = nc.dram_tensor("masktok", [NT2], F32, kind="ExternalInput")
    w0 = nc.dram_tensor("w0", [L, BE], F32, kind="ExternalInput")
    bv0 = nc.dram_tensor("bv0", [L, BE], F32, kind="ExternalInput")
    onehotT = nc.dram_tensor("onehotT", [L, NT2], BF16, kind="ExternalInput")
    logitsT_d = nc.dram_tensor("logitsT", [L, NT2], F32, kind="ExternalOutput")
    lnS_d = nc.dram_tensor("lnS", [1, BE], F32, kind="ExternalOutput")
    accF_d = nc.dram_tensor("accF", [BE, 1], F32, kind="ExternalOutput")
    accB_d = nc.dram_tensor("accB", [BE, 1], F32, kind="ExternalOutput")
    bil_d = nc.dram_tensor("bil", [1, BE], F32, kind="ExternalOutput")

    with tile.TileContext(nc) as tc:
        with tc.tile_pool(name="w2", bufs=1) as wpool, \
             tc.tile_pool(name="wk2", bufs=2) as work:
            ident = wpool.tile([128, 128], F32)
            make_identity(nc, ident[:])
            o_sb = wpool.tile([128, 8, NT2], BF16)
            nc.sync.dma_start(o_sb[:], oT[:].rearrange("k p t e -> p k (t e)"))
            wout_sb = wpool.tile([128, 8, L], BF16)
            nc.sync.dma_start(wout_sb[:], WoutT[:].rearrange("(k p) m -> p k m", p=128))
            bout_sb = wpool.tile([L, 1], F32)
            nc.sync.dma_start(bout_sb[:], boutT[:])
            ehf_sb = wpool.tile([L, L], F32)
            nc.sync.dma_start(ehf_sb[:], EhatTf[:])
            ehb_sb = wpool.tile([L, L], F32)
            nc.sync.dma_start(ehb_sb[:], EhatB[:])
            mf_sb = wpool.tile([L, S_HALF, BE], I32)
            nc.sync.dma_start(mf_sb[:], bass.AP(
                tensor=maskF[:].tensor, offset=0,
                ap=[[0, L], [BE, S_HALF], [1, BE]]))
            mb_sb = wpool.tile([L, S_HALF, BE], I32)
            nc.sync.dma_start(mb_sb[:], bass.AP(
                tensor=maskB[:].tensor, offset=0,
                ap=[[0, L], [BE, S_HALF], [1, BE]]))
            mtok_sb = wpool.tile([L, NT2], F32)
            nc.sync.dma_start(mtok_sb[:], bass.AP(
                tensor=masktok[:].tensor, offset=0, ap=[[0, L], [1, NT2]]))
            oh_sb = wpool.tile([L, NT2], BF16)
            nc.sync.dma_start(oh_sb[:], onehotT[:])
            w_sb = wpool.tile([L, BE], F32)
            nc.sync.dma_start(w_sb[:], w0[:])
            bv_sb = wpool.tile([L, BE], F32)
            nc.sync.dma_start(bv_sb[:], bv0[:])
            ones_sb = wpool.tile([L, 1], F32)
            nc.vector.memset(ones_sb[:], 1.0)
            accF_sb = wpool.tile([BE, 1], F32)
            nc.vector.memset(accF_sb[:], 0.0)
            accB_sb = wpool.tile([BE, 1], F32)
            nc.vector.memset(accB_sb[:], 0.0)
            lgT_sb = wpool.tile([L, NT2], F32)
            elog_sb = wpool.tile([L, NT2], F32)

            # ---- logits ----
            psA_ctx = tc.tile_pool(name="psA", bufs=2, space="PSUM")
            psA = psA_ctx.__enter__()
            for tsb in range(NT2 // 512):
                ts_ = slice(tsb * 512, (tsb + 1) * 512)
                ps = psA.tile([L, 512], F32, tag="lg")
                for k in range(8):
                    nc.tensor.matmul(ps[:], wout_sb[:, k, :], o_sb[:, k, ts_],
                                     start=(k == 0), stop=(k == 7))
                bcol = bout_sb[:, 0:1]
                tmin = work.tile([L, 512], F32, tag="tmin")
                nc.vector.tensor_scalar(out=tmin[:], in0=ps[:], scalar1=bcol,
                                        op0=ALU.add, scalar2=0.0, op1=ALU.min)
                pp = work.tile([L, 512], F32, tag="pp")
                nc.vector.scalar_tensor_tensor(out=pp[:], in0=ps[:], scalar=bcol,
                                               op0=ALU.add, in1=tmin[:],
                                               op1=ALU.subtract)
                ee = work.tile([L, 512], F32, tag="ee")
                nc.scalar.activation(ee[:], tmin[:], AF.Exp)
                nc.vector.tensor_scalar(out=ee[:], in0=ee[:], scalar1=1.0,
                                        op0=ALU.subtract, scalar2=SELU_SA,
                                        op1=ALU.mult)
                sl = work.tile([L, 512], F32, tag="sl")
                nc.vector.scalar_tensor_tensor(out=sl[:], in0=pp[:], scalar=SELU_S,
                                               op0=ALU.mult, in1=ee[:], op1=ALU.add)
                nc.vector.tensor_mul(lgT_sb[:, ts_], sl[:], mtok_sb[:, ts_])
                nc.scalar.activation(elog_sb[:, ts_], lgT_sb[:, ts_], AF.Exp)
            nc.sync.dma_start(logitsT_d[:], lgT_sb[:])

            # ---- emission score ----
            p2 = work.tile([L, NT2], F32, tag="p2")
            nc.vector.tensor_mul(p2[:], oh_sb[:], lgT_sb[:])
            bil_red = work.tile([L, BE], F32, tag="bilred")
            nc.vector.tensor_reduce(
                out=bil_red[:], in_=p2[:].rearrange("l (t e) -> l e t", e=BE),
                axis=mybir.AxisListType.X, op=ALU.add)
            bps = psA.tile([1, BE], F32, tag="bil")
            nc.tensor.matmul(bps[:], ones_sb[:], bil_red[:], start=True, stop=True)
            bil_sb = work.tile([1, BE], F32, tag="bilsb")
            nc.vector.tensor_copy(bil_sb[:], bps[:])
            nc.sync.dma_start(bil_d[:], bil_sb[:])

            psA_ctx.__exit__(None, None, None)

            # ---- CRF: interleaved fwd/bwd scaled chains ----
            psB_ctx = tc.tile_pool(name="psB", bufs=2, space="PSUM")
            psB = psB_ctx.__enter__()
            psC_ctx = tc.tile_pool(name="psC", bufs=1, space="PSUM")
            psC = psC_ctx.__enter__()
            elog3 = elog_sb[:].rearrange("l (t e) -> l t e", e=BE)
            for s in range(S_HALF):
                tf = s
                tb = T - 1 - s
                vf = psB.tile([L, BE], F32, tag="vf")
                nc.tensor.matmul(vf[:], ehf_sb[:], w_sb[:], start=True, stop=True)
                uf = work.tile([L, BE], F32, tag="uf")
                nc.vector.tensor_mul(uf[:], vf[:], elog3[:, tf, :])
                nc.vector.copy_predicated(w_sb[:], mf_sb[:, s, :], uf[:])

                ub = work.tile([L, BE], F32, tag="ub")
                nc.vector.tensor_mul(ub[:], bv_sb[:], elog3[:, tb, :])
                vb = psB.tile([L, BE], F32, tag="vb")
                nc.tensor.matmul(vb[:], ehb_sb[:], ub[:], start=True, stop=True)
                nc.vector.copy_predicated(bv_sb[:], mb_sb[:, s, :], vb[:])

                if s % 32 == 31:
                    for chain, acc in ((w_sb, accF_sb), (bv_sb, accB_sb)):
                        trp = psC.tile([BE, L], F32, tag="trp")
                        nc.tensor.transpose(trp[:], chain[:], ident[0:L, 0:L])
                        mx = work.tile([BE, 1], F32, tag="mx")
                        nc.vector.tensor_reduce(out=mx[:], in_=trp[:],
                                                axis=mybir.AxisListType.X,
                                                op=ALU.max)
                        lnv = work.tile([BE, 1], F32, tag="lnv")
                        nc.scalar.activation(lnv[:], mx[:], AF.Ln)
                        nc.vector.tensor_add(acc[:], acc[:], lnv[:])
                        rc = work.tile([BE, 1], F32, tag="rc")
                        nc.vector.reciprocal(rc[:], mx[:])
                        wts = work.tile([BE, L], F32, tag="wts")
                        nc.vector.tensor_scalar_mul(out=wts[:], in0=trp[:],
                                                    scalar1=rc[:])
                        wps = psC.tile([L, BE], F32, tag="wps")
                        nc.tensor.transpose(wps[:], wts[:], ident[0:BE, 0:BE])
                        nc.vector.tensor_copy(chain[:], wps[:])

            pz = work.tile([L, BE], F32, tag="pz")
            nc.vector.tensor_mul(pz[:], w_sb[:], bv_sb[:])
            zp = psC.tile([1, BE], F32, tag="zp")
            nc.tensor.matmul(zp[:], ones_sb[:], pz[:], start=True, stop=True)
            lnS_sb = work.tile([1, BE], F32, tag="lns")
            nc.scalar.activation(lnS_sb[:], zp[:], AF.Ln)
            nc.sync.dma_start(lnS_d[:], lnS_sb[:])
            nc.sync.dma_start(accF_d[:], accF_sb[:])
            nc.sync.dma_start(accB_d[:], accB_sb[:])
            psC_ctx.__exit__(None, None, None)
            psB_ctx.__exit__(None, None, None)
    return nc


# ---------------------------------------------------------------------------
# host orchestration
# ---------------------------------------------------------------------------
_CACHE = {}


def _get(name, builder):
    if name not in _CACHE:
        _CACHE[name] = builder()
    return _CACHE[name]


def _selu_host(x):
    neg = np.minimum(x, 0.0)
    return SELU_S * (x - neg) + SELU_SA * (np.exp(neg) - 1.0)


def kernel(xs, y, lens, emb0, emb1, W_in, b_in, W_ih_f, W_hh_f, b_f,
           W_ih_b, W_hh_b, b_b, W_out, b_out, trans):
    xs = np.asarray(xs)
    y = np.asarray(y)
    lens_in = np.asarray(lens)
    lens64 = lens_in.astype(np.int64)
    emb0 = f32(emb0); emb1 = f32(emb1)
    W_in = f32(W_in); b_in = f32(b_in)
    W_ih_f = f32(W_ih_f); W_hh_f = f32(W_hh_f); b_f = f32(b_f)
    W_ih_b = f32(W_ih_b); W_hh_b = f32(W_hh_b); b_b = f32(b_b)
    W_out = f32(W_out); b_out = f32(b_out); trans = f32(trans)

    mask = (np.arange(T)[None, :] < lens64[:, None]).astype(np.float32)  # [B,T]

    # ---------------- K1 ----------------
    nc1 = _get("k1", build_k1)
    binT_h = b_in.reshape(4, 128).T.copy()
    in_maps = []
    for c in range(NCORES):
        d = c // 4
        q = c % 4
        ex = slice(q * BQ, (q + 1) * BQ)
        xs_q = xs[:, ex, :]
        mk_q = mask[ex, :]
        if d == 1:
            xs_q = xs_q[:, :, ::-1]
            mk_q = mk_q[:, ::-1]
        # t-major flatten: flat[i] with i = t*BQ + e
        tokf = np.ascontiguousarray(xs_q.transpose(0, 2, 1)).reshape(2, NTOK)
        mflat = np.ascontiguousarray(mk_q.T).reshape(NTOK).astype(np.float32)
        toksw = tokf.reshape(2, NCH, 128).transpose(0, 2, 1)  # [2,128,NCH]
        W_ih = W_ih_f if d == 0 else W_ih_b
        W_hh = W_hh_f if d == 0 else W_hh_b
        bb = b_f if d == 0 else b_b
        in_maps.append({
            "tok0": np.ascontiguousarray(toksw[0]).astype(np.int32),
            "tok1": np.ascontiguousarray(toksw[1]).astype(np.int32),
            "maskflat": mflat,
            "emb0": emb0, "emb1": emb1,
            "WinT": bf16(W_in.T),
            "WihT": bf16(W_ih.T),
            "WhhT": bf16(W_hh.T),
            "binT": binT_h,
            "bihT": bb.reshape(16, 128).T.copy(),
        })
    res1 = bass_utils.run_bass_kernel_spmd(nc1, in_maps, core_ids=list(range(NCORES)))
    # hsT per core: [4, 128, T, BQ] bf16
    hs = [r["hsT"] for r in res1.results]

    # ---------------- regroup for K2 ----------------
    # o chunks 0-3 = h_f, 4-7 = h_b (bwd cores' hsT are in reversed time)
    nc2 = _get("k2", build_k2)
    Ehat = np.exp(trans) / math.exp(CLOG)
    NT2 = T * BE
    in_maps2 = []
    for c2 in range(NCORES):
        q, half = divmod(c2, 2)
        ex = slice(q * BQ + half * BE, q * BQ + (half + 1) * BE)
        esub = slice(half * BE, (half + 1) * BE)
        hf = hs[q][:, :, :, esub]                      # [4,128,T,BE]
        hb = hs[q + 4][:, :, ::-1, esub]               # time-flip back
        oT_h = np.concatenate([hf, hb], axis=0)        # [8,128,T,BE] bf16
        mk = mask[ex, :]                               # [BE,T]
        y_q = y[ex, :].astype(np.int64)
        oh = np.zeros((L, NT2), np.float32)
        ti = np.repeat(np.arange(T), BE) * BE + np.tile(np.arange(BE), T)
        oh[y_q.T.reshape(-1), ti] = mk.T.reshape(-1)
        w0_h = np.zeros((L, BE), np.float32)
        w0_h[START, :] = 1.0
        bv0_h = np.repeat(np.exp(trans[STOP])[:, None], BE, 1).astype(np.float32)
        in_maps2.append({
            "oT": np.ascontiguousarray(oT_h),
            "WoutT": bf16(W_out.T),
            "boutT": b_out.reshape(L, 1).copy(),
            "EhatTf": f32(Ehat.T),
            "EhatB": f32(Ehat),
            "maskF": np.ascontiguousarray(mk.T[:S_HALF]).astype(np.int32),
            "maskB": np.ascontiguousarray(mk.T[::-1][:S_HALF]).astype(np.int32),
            "masktok": np.ascontiguousarray(mk.T).reshape(NT2),
            "w0": w0_h,
            "bv0": bv0_h,
            "onehotT": bf16(oh),
        })
    res2 = bass_utils.run_bass_kernel_spmd(nc2, in_maps2, core_ids=list(range(NCORES)))

    # ---------------- host assembly ----------------
    logits = np.zeros((B, T, L), np.float32)
    loglik = np.zeros(B, np.float32)
    # transition score on host (pure input gather)
    raw = np.concatenate([np.full((B, 1), START, np.int64),
                          y.astype(np.int64),
                          np.full((B, 1), STOP, np.int64)], 1)
    m2 = np.arange(T + 2)[None, :] < (lens64 + 1)[:, None]
    labels = np.where(m2, raw, STOP)
    trn = trans[labels[:, 1:], labels[:, :-1]]
    m3 = (np.arange(T + 1)[None, :] < (lens64 + 1)[:, None]).astype(np.float32)
    trans_score = (trn * m3).sum(1)

    for c2 in range(NCORES):
        q, half = divmod(c2, 2)
        ex = slice(q * BQ + half * BE, q * BQ + (half + 1) * BE)
        r = res2.results[c2]
        lgT = r["logitsT"].reshape(L, T, BE)
        logits[ex] = lgT.transpose(2, 1, 0)
        logZ = (r["lnS"][0] + r["accF"][:, 0] + r["accB"][:, 0]
                + lens64[ex] * CLOG)
        loglik[ex] = trans_score[ex] + r["bil"][0] - logZ

    return loglik, logits
